# revision 4
# baseline (speedup 1.0000x reference)
"""BitNet linear layer (b1.58-style) on 8 Trainium2 NeuronCores.

Computes: scale = 1e-4 + mean(|W|); q = clip(round(W/scale), -1, 1);
          out = scale * (x @ q.T)
for x [4, 2048, 2048] f32 and W [8192, 2048] f32.

Sharding: tensor-parallel over out_features. Each core gets the full x
(replicated) and a 1024-row shard of W; host concatenates the 8 per-core
[8192, 1024] outputs along the feature axis.

On-device per core:
  - W pass 1: stream shard, |W| row-sums (DVE), partition all-reduce
    (GPSIMD), 4-byte AllReduce across the 8 cores -> global scale.
  - W pass 2: quantize via threshold compares (q = (W > .5s) - (W < -.5s),
    exactly equivalent to clip(round(W/s)) for |W/s| < 2.5 with
    round-half-even boundary behavior), cast to bf16, transpose 128x128
    tiles on the PE (regular matmul against an identity) into qT [K, N].
  - x pipeline: DMA 128-token tiles (f32), cast to bf16 (DVE), PE-transpose
    into xT [K, 128], then accumulate out[m, n] = sum_k xT[k, m] * qT[k, n]
    in PSUM over 16 k-tiles; ACT copies PSUM->SBUF fused with *scale; DMA out.
"""

import os
import sys

sys.path.insert(0, "/opt/trn_rl_repo")

import numpy as np

import concourse.bass as bass
import concourse.tile as tile
from concourse import bacc, mybir
from concourse.bass_utils import run_bass_kernel_spmd
from concourse.masks import make_identity
from concourse import bass_isa

F32 = mybir.dt.float32
BF16 = mybir.dt.bfloat16

NCORES = 8
M = 8192          # tokens (4*2048)
K = 2048          # in_features
N_FULL = 8192     # out_features
NS = N_FULL // NCORES  # 1024 per-core shard
P = 128
KO = K // P       # 16 k-tiles
NO = NS // P      # 8 W-row tiles per shard
MT = M // P       # 64 m-tiles
W_ELEMS = float(N_FULL * K)  # 16777216, for the mean

PREFIX = 10       # m-tiles of x work emitted before the scale collective


def build_nc():
    nc = bacc.Bacc("TRN2", target_bir_lowering=False, debug=False,
                   num_devices=NCORES)
    x_d = nc.dram_tensor("x", [M, K], F32, kind="ExternalInput")
    w_d = nc.dram_tensor("w", [NS, K], F32, kind="ExternalInput")
    o_d = nc.dram_tensor("out", [M, NS], F32, kind="ExternalOutput")
    x_ap, w_ap, o_ap = x_d.ap(), w_d.ap(), o_d.ap()

    with tile.TileContext(nc) as tc:
        with (
            tc.tile_pool(name="const", bufs=1) as const,
            tc.tile_pool(name="scal", bufs=1) as scal,
            tc.tile_pool(name="wpool", bufs=3) as wpool,
            tc.tile_pool(name="gpool", bufs=1) as gpool,
            tc.tile_pool(name="qtpool", bufs=2) as qtpool,
            tc.tile_pool(name="qT_pool", bufs=1) as qT_pool,
            tc.tile_pool(name="xpool", bufs=3) as xpool,
            tc.tile_pool(name="xbpool", bufs=3) as xbpool,
            tc.tile_pool(name="xTpool", bufs=PREFIX + 2) as xTpool,
            tc.tile_pool(name="opool", bufs=4) as opool,
            tc.tile_pool(name="psum_t", bufs=3, space="PSUM") as psum_t,
            tc.tile_pool(name="psum_o", bufs=4, space="PSUM") as psum_o,
            tc.tile_pool(name="dram", bufs=1, space="DRAM") as dram,
        ):
            ident = const.tile([P, P], BF16, name="ident")
            make_identity(nc, ident)

            # ---- W pass 1: global scale -------------------------------
            wabs = scal.tile([P, NO], F32, name="wabs")
            for o in range(NO):
                wt = wpool.tile([P, K], F32, name=f"w1_{o}", tag="w")
                nc.gpsimd.dma_start(wt[:], w_ap[o * P:(o + 1) * P, :])
                nc.vector.tensor_reduce(
                    wabs[:, o:o + 1], wt[:], mybir.AxisListType.X,
                    mybir.AluOpType.add, apply_absolute_value=True)
            wsum = scal.tile([P, 1], F32, name="wsum")
            nc.vector.tensor_reduce(
                wsum[:], wabs[:], mybir.AxisListType.X, mybir.AluOpType.add)
            tot128 = scal.tile([P, 1], F32, name="tot128")
            nc.gpsimd.partition_all_reduce(
                tot128[:], wsum[:], P, bass_isa.ReduceOp.add)

            cc_in = dram.tile([1, 1], F32, name="cc_in")
            cc_out = dram.tile([1, 1], F32, name="cc_out", addr_space="Shared")
            nc.gpsimd.dma_start(cc_in[:], tot128[0:1, :])
            nc.gpsimd.collective_compute(
                "AllReduce", mybir.AluOpType.add,
                replica_groups=[list(range(NCORES))],
                ins=[cc_in[:].opt()], outs=[cc_out[:].opt()])
            tot_sb = scal.tile([1, 1], F32, name="tot_sb")
            nc.gpsimd.dma_start(tot_sb[:], cc_out[:])
            bcast = scal.tile([P, 1], F32, name="bcast")
            nc.gpsimd.partition_broadcast(bcast[:], tot_sb[:])

            # thr = 0.5*scale = 0.5e-4 + tot/(2*W_ELEMS);  scale = 1e-4 + tot/W_ELEMS
            thr_pos = scal.tile([P, 1], F32, name="thr_pos")
            nc.vector.tensor_scalar(
                thr_pos[:], bcast[:], 0.5 / W_ELEMS, 0.5e-4,
                mybir.AluOpType.mult, mybir.AluOpType.add)
            thr_neg = scal.tile([P, 1], F32, name="thr_neg")
            nc.vector.tensor_scalar(
                thr_neg[:], thr_pos[:], -1.0, None, mybir.AluOpType.mult)
            scale_col = scal.tile([P, 1], F32, name="scale_col")
            nc.vector.tensor_scalar(
                scale_col[:], bcast[:], 1.0 / W_ELEMS, 1e-4,
                mybir.AluOpType.mult, mybir.AluOpType.add)

            # ---- x pipeline stage -------------------------------------
            def x_stage(mt):
                xt = xpool.tile([P, K], F32, name=f"x_{mt}", tag="x")
                nc.sync.dma_start(xt[:], x_ap[mt * P:(mt + 1) * P, :])
                xb = xbpool.tile([P, K], BF16, name=f"xb_{mt}", tag="xb")
                nc.vector.tensor_copy(xb[:], xt[:])
                xT = xTpool.tile([P, KO, P], BF16, name=f"xT_{mt}", tag="xT")
                for g in range(4):
                    pt = psum_t.tile([P, 4 * P], F32, name=f"ptx_{mt}_{g}",
                                     tag="pt")
                    for j in range(4):
                        ko = g * 4 + j
                        nc.tensor.matmul(
                            pt[:, j * P:(j + 1) * P],
                            lhsT=xb[:, ko * P:(ko + 1) * P],
                            rhs=ident[:], start=True, stop=True)
                    nc.scalar.activation(
                        xT[:, g * 4:(g + 1) * 4, :], pt[:],
                        mybir.ActivationFunctionType.Copy)
                return xT

            prefix_xT = {mt: x_stage(mt) for mt in range(PREFIX)}

            # ---- W pass 2: quantize + transpose -> qT [P, KO, NS] -----
            qT = qT_pool.tile([P, KO, NS], BF16, name="qT")
            for o in range(NO):
                wt2 = wpool.tile([P, K], F32, name=f"w2_{o}", tag="w")
                nc.gpsimd.dma_start(wt2[:], w_ap[o * P:(o + 1) * P, :])
                ga = gpool.tile([P, K], F32, name=f"ga_{o}", tag="ga")
                nc.vector.tensor_scalar(
                    ga[:], wt2[:], thr_pos[:], None, mybir.AluOpType.is_gt)
                gb = gpool.tile([P, K], F32, name=f"gb_{o}", tag="gb")
                nc.vector.tensor_scalar(
                    gb[:], wt2[:], thr_neg[:], None, mybir.AluOpType.is_lt)
                qt = qtpool.tile([P, K], BF16, name=f"qt_{o}", tag="qt")
                nc.vector.tensor_tensor(
                    qt[:], ga[:], gb[:], mybir.AluOpType.subtract)
                for g in range(4):
                    pt = psum_t.tile([P, 4 * P], F32, name=f"ptq_{o}_{g}",
                                     tag="pt")
                    for j in range(4):
                        ko = g * 4 + j
                        nc.tensor.matmul(
                            pt[:, j * P:(j + 1) * P],
                            lhsT=qt[:, ko * P:(ko + 1) * P],
                            rhs=ident[:], start=True, stop=True)
                    nc.scalar.activation(
                        qT[:, g * 4:(g + 1) * 4, o * P:(o + 1) * P],
                        pt[:].rearrange("p (a b) -> p a b", a=4),
                        mybir.ActivationFunctionType.Copy)

            # ---- main loop: matmul + scale + store --------------------
            for mt in range(MT):
                xT = prefix_xT.pop(mt) if mt in prefix_xT else x_stage(mt)
                ot = opool.tile([P, NS], F32, name=f"o_{mt}", tag="o")
                for nh in range(2):
                    po = psum_o.tile([P, 512], F32, name=f"po_{mt}_{nh}",
                                     tag="po")
                    for ko in range(KO):
                        nc.tensor.matmul(
                            po[:],
                            lhsT=xT[:, ko, :],
                            rhs=qT[:, ko, nh * 512:(nh + 1) * 512],
                            start=(ko == 0), stop=(ko == KO - 1))
                    nc.scalar.activation(
                        ot[:, nh * 512:(nh + 1) * 512], po[:],
                        mybir.ActivationFunctionType.Copy, scale=scale_col[:])
                nc.sync.dma_start(o_ap[mt * P:(mt + 1) * P, :], ot[:])

    nc.compile()
    return nc


_NC_CACHE = None


def get_nc():
    global _NC_CACHE
    if _NC_CACHE is None:
        _NC_CACHE = build_nc()
    return _NC_CACHE


def make_in_maps(x, weight):
    x2 = np.ascontiguousarray(np.asarray(x, dtype=np.float32).reshape(M, K))
    w = np.asarray(weight, dtype=np.float32)
    return [
        {"x": x2, "w": np.ascontiguousarray(w[c * NS:(c + 1) * NS])}
        for c in range(NCORES)
    ]


def kernel(x, weight):
    nc = get_nc()
    in_maps = make_in_maps(x, weight)
    try:
        res = run_bass_kernel_spmd(nc, in_maps, list(range(NCORES)))
    except Exception:
        # transient device errors have been observed on first touch; retry once
        res = run_bass_kernel_spmd(nc, in_maps, list(range(NCORES)))
    out = np.concatenate(
        [res.results[c]["out"] for c in range(NCORES)], axis=1)
    return np.ascontiguousarray(out.reshape(4, 2048, N_FULL), dtype=np.float32)


# revision 8
# speedup vs baseline: 1.0163x; 1.0163x over previous
"""BitNet linear layer (b1.58-style) on 8 Trainium2 NeuronCores.

Computes: scale = 1e-4 + mean(|W|); q = clip(round(W/scale), -1, 1);
          out = scale * (x @ q.T)
for x [4, 2048, 2048] f32 and W [8192, 2048] f32.

Sharding: tensor-parallel over out_features. Each core gets the full x
(replicated) and a 1024-row shard of W; host concatenates the 8 per-core
[8192, 1024] outputs along the feature axis.

On-device per core:
  - W pass 1 (HWDGE/sync ring, first in line): stream shard, |W| row-sums
    (DVE), partition all-reduce (GPSIMD), 4-byte AllReduce across the 8
    cores (bounce DMAs on the ACT HWDGE ring so they never queue behind
    bulk transfers) -> global scale.
  - W pass 2: quantize via mask = (|W| > 0.5*scale) and sign(W) (exactly
    equivalent to clip(round(W/s)) for |W/s| < 2.5 with round-half-even
    boundary behavior), cast to bf16, transpose 128x128 tiles on the PE
    (regular matmul against an identity) into qT [K, N].
  - x pipeline: SWDGE cast-DMA 128-token tiles (f32 in HBM -> bf16 in
    SBUF), PE-transpose into xT [K, 128] (DVE drains PSUM), then
    accumulate out[m, n] = sum_k xT[k, m] * qT[k, n] in PSUM, k-outer so
    each stationary xT[k] serves the full 1024-wide moving slice of qT;
    ACT copies PSUM->SBUF fused with *scale; DMA out.
"""

import os
import sys

sys.path.insert(0, "/opt/trn_rl_repo")

import numpy as np

import concourse.bass as bass
import concourse.tile as tile
from concourse import bacc, mybir
from concourse.bass_utils import run_bass_kernel_spmd
from concourse.masks import make_identity
from concourse import bass_isa

F32 = mybir.dt.float32
BF16 = mybir.dt.bfloat16

NCORES = 8
M = 8192          # tokens (4*2048)
K = 2048          # in_features
N_FULL = 8192     # out_features
NS = N_FULL // NCORES  # 1024 per-core shard
P = 128
KO = K // P       # 16 k-tiles
NO = NS // P      # 8 W-row tiles per shard
MT = M // P       # 64 m-tiles
W_ELEMS = float(N_FULL * K)  # 16777216, for the mean

PREFIX = 16       # m-tiles of x work emitted before the scale collective
N_MOVE = 512      # moving free dim per accumulation matmul (1024 rejected by ISA check)


def build_nc():
    nc = bacc.Bacc("TRN2", target_bir_lowering=False, debug=False,
                   num_devices=NCORES)
    x_d = nc.dram_tensor("x", [M, K], F32, kind="ExternalInput")
    w_d = nc.dram_tensor("w", [NS, K], F32, kind="ExternalInput")
    o_d = nc.dram_tensor("out", [M, NS], F32, kind="ExternalOutput")
    x_ap, w_ap, o_ap = x_d.ap(), w_d.ap(), o_d.ap()
    NH = NS // N_MOVE

    with tile.TileContext(nc) as tc:
        with (
            tc.tile_pool(name="const", bufs=1) as const,
            tc.tile_pool(name="scal", bufs=1) as scal,
            tc.tile_pool(name="wpool", bufs=3) as wpool,
            tc.tile_pool(name="gpool", bufs=2) as gpool,
            tc.tile_pool(name="qtpool", bufs=2) as qtpool,
            tc.tile_pool(name="qT_pool", bufs=1) as qT_pool,
            tc.tile_pool(name="xbpool", bufs=4) as xbpool,
            tc.tile_pool(name="xTpool", bufs=PREFIX + 2) as xTpool,
            tc.tile_pool(name="opool", bufs=4) as opool,
            tc.tile_pool(name="psum_t", bufs=3, space="PSUM") as psum_t,
            tc.tile_pool(name="psum_o", bufs=2 * NH, space="PSUM") as psum_o,
            tc.tile_pool(name="dram", bufs=1, space="DRAM") as dram,
        ):
            ident = const.tile([P, P], BF16, name="ident")
            make_identity(nc, ident)

            # ---- W pass 1: global scale -------------------------------
            wabs = scal.tile([P, NO], F32, name="wabs")
            for o in range(NO):
                wt = wpool.tile([P, K], F32, name=f"w1_{o}", tag="w")
                nc.sync.dma_start(wt[:], w_ap[o * P:(o + 1) * P, :])
                nc.vector.tensor_reduce(
                    wabs[:, o:o + 1], wt[:], mybir.AxisListType.X,
                    mybir.AluOpType.add, apply_absolute_value=True)
            wsum = scal.tile([P, 1], F32, name="wsum")
            nc.vector.tensor_reduce(
                wsum[:], wabs[:], mybir.AxisListType.X, mybir.AluOpType.add)
            tot128 = scal.tile([P, 1], F32, name="tot128")
            nc.gpsimd.partition_all_reduce(
                tot128[:], wsum[:], P, bass_isa.ReduceOp.add)

            cc_in = dram.tile([1, 1], F32, name="cc_in")
            cc_out = dram.tile([1, 1], F32, name="cc_out", addr_space="Shared")
            nc.scalar.dma_start(cc_in[:], tot128[0:1, :])
            nc.gpsimd.collective_compute(
                "AllReduce", mybir.AluOpType.add,
                replica_groups=[list(range(NCORES))],
                ins=[cc_in[:].opt()], outs=[cc_out[:].opt()])
            tot_sb = scal.tile([1, 1], F32, name="tot_sb")
            nc.scalar.dma_start(tot_sb[:], cc_out[:])
            bcast = scal.tile([P, 1], F32, name="bcast")
            nc.gpsimd.partition_broadcast(bcast[:], tot_sb[:])

            # thr = 0.5*scale = 0.5e-4 + tot/(2*W_ELEMS); scale = 1e-4 + tot/W_ELEMS
            thr_pos = scal.tile([P, 1], F32, name="thr_pos")
            nc.vector.tensor_scalar(
                thr_pos[:], bcast[:], 0.5 / W_ELEMS, 0.5e-4,
                mybir.AluOpType.mult, mybir.AluOpType.add)
            thr_neg = scal.tile([P, 1], F32, name="thr_neg")
            nc.vector.tensor_scalar(
                thr_neg[:], thr_pos[:], -1.0, None, mybir.AluOpType.mult)
            scale_col = scal.tile([P, 1], F32, name="scale_col")
            nc.vector.tensor_scalar(
                scale_col[:], bcast[:], 1.0 / W_ELEMS, 1e-4,
                mybir.AluOpType.mult, mybir.AluOpType.add)

            # ---- x pipeline stage -------------------------------------
            def x_stage(mt):
                xb = xbpool.tile([P, K], BF16, name=f"xb_{mt}", tag="xb")
                # SWDGE cast-DMA: f32 HBM -> bf16 SBUF
                nc.gpsimd.dma_start(xb[:], x_ap[mt * P:(mt + 1) * P, :])
                xT = xTpool.tile([P, KO, P], BF16, name=f"xT_{mt}", tag="xT")
                for g in range(4):
                    pt = psum_t.tile([P, 4 * P], F32, name=f"ptx_{mt}_{g}",
                                     tag="pt")
                    for j in range(4):
                        ko = g * 4 + j
                        nc.tensor.matmul(
                            pt[:, j * P:(j + 1) * P],
                            lhsT=xb[:, ko * P:(ko + 1) * P],
                            rhs=ident[:], start=True, stop=True)
                    nc.vector.tensor_copy(xT[:, g * 4:(g + 1) * 4, :], pt[:])
                return xT

            prefix_xT = {mt: x_stage(mt) for mt in range(PREFIX)}

            # ---- W pass 2: quantize + transpose -> qT [P, KO, NS] -----
            qT = qT_pool.tile([P, KO, NS], BF16, name="qT")
            for o in range(NO):
                wt2 = wpool.tile([P, K], F32, name=f"w2_{o}", tag="w")
                nc.sync.dma_start(wt2[:], w_ap[o * P:(o + 1) * P, :])
                # q = (w > thr) - (w < -thr)  ==  clip(round(w/scale), -1, 1)
                ga = gpool.tile([P, K], BF16, name=f"ga_{o}", tag="ga")
                nc.vector.tensor_scalar(
                    ga[:], wt2[:], thr_pos[:], None, mybir.AluOpType.is_gt)
                gb = gpool.tile([P, K], BF16, name=f"gb_{o}", tag="gb")
                nc.vector.tensor_scalar(
                    gb[:], wt2[:], thr_neg[:], None, mybir.AluOpType.is_lt)
                qt = qtpool.tile([P, K], BF16, name=f"qt_{o}", tag="qt")
                nc.vector.tensor_tensor(
                    qt[:], ga[:], gb[:], mybir.AluOpType.subtract)
                for g in range(4):
                    pt = psum_t.tile([P, 4 * P], F32, name=f"ptq_{o}_{g}",
                                     tag="pt")
                    for j in range(4):
                        ko = g * 4 + j
                        nc.tensor.matmul(
                            pt[:, j * P:(j + 1) * P],
                            lhsT=qt[:, ko * P:(ko + 1) * P],
                            rhs=ident[:], start=True, stop=True)
                    nc.scalar.activation(
                        qT[:, g * 4:(g + 1) * 4, o * P:(o + 1) * P],
                        pt[:].rearrange("p (a b) -> p a b", a=4),
                        mybir.ActivationFunctionType.Copy)

            # ---- main loop: matmul + scale + store --------------------
            for mt in range(MT):
                xT = prefix_xT.pop(mt) if mt in prefix_xT else x_stage(mt)
                ot = opool.tile([P, NS], F32, name=f"o_{mt}", tag="o")
                pos = [psum_o.tile([P, N_MOVE], F32, name=f"po_{mt}_{nh}",
                                   tag="po") for nh in range(NH)]
                for ko in range(KO):
                    for nh in range(NH):
                        nc.tensor.matmul(
                            pos[nh][:],
                            lhsT=xT[:, ko, :],
                            rhs=qT[:, ko, nh * N_MOVE:(nh + 1) * N_MOVE],
                            start=(ko == 0), stop=(ko == KO - 1))
                for nh in range(NH):
                    nc.scalar.activation(
                        ot[:, nh * N_MOVE:(nh + 1) * N_MOVE], pos[nh][:],
                        mybir.ActivationFunctionType.Copy, scale=scale_col[:])
                nc.sync.dma_start(o_ap[mt * P:(mt + 1) * P, :], ot[:])

    nc.compile()
    return nc


_NC_CACHE = None


def get_nc():
    global _NC_CACHE
    if _NC_CACHE is None:
        _NC_CACHE = build_nc()
    return _NC_CACHE


def make_in_maps(x, weight):
    x2 = np.ascontiguousarray(np.asarray(x, dtype=np.float32).reshape(M, K))
    w = np.asarray(weight, dtype=np.float32)
    return [
        {"x": x2, "w": np.ascontiguousarray(w[c * NS:(c + 1) * NS])}
        for c in range(NCORES)
    ]


def kernel(x, weight):
    nc = get_nc()
    in_maps = make_in_maps(x, weight)
    try:
        res = run_bass_kernel_spmd(nc, in_maps, list(range(NCORES)))
    except Exception:
        # transient device errors have been observed on first touch; retry once
        res = run_bass_kernel_spmd(nc, in_maps, list(range(NCORES)))
    out = np.concatenate(
        [res.results[c]["out"] for c in range(NCORES)], axis=1)
    return np.ascontiguousarray(out.reshape(4, 2048, N_FULL), dtype=np.float32)


# revision 12
# speedup vs baseline: 1.0357x; 1.0190x over previous
"""BitNet linear layer (b1.58-style) on 8 Trainium2 NeuronCores.

Computes: scale = 1e-4 + mean(|W|); q = clip(round(W/scale), -1, 1);
          out = scale * (x @ q.T)
for x [4, 2048, 2048] f32 and W [8192, 2048] f32.

Sharding: tensor-parallel over out_features. Each core gets the full x
(replicated) and a 1024-row shard of W; host concatenates the 8 per-core
[8192, 1024] outputs along the feature axis.

On-device per core:
  - W pass 1 (sync/HWDGE ring, right after a 2-tile x head): stream the
    shard, |W| row-sums (DVE), partition all-reduce (GPSIMD), 4-byte
    AllReduce across the 8 cores (all collective plumbing is alone on the
    GPSIMD ring so it never queues behind bulk DMAs) -> global scale.
  - Quantize: q = (W > .5*scale) - (W < -.5*scale), exactly equal to
    clip(round(W/s), -1, 1) for |W/s| < 2.5 with round-half-even boundary
    behavior; bf16 result is transposed 128x128 on the PE (regular matmul
    against an identity) into qT [K, N]. W tiles are mostly kept resident
    from pass 1 (bufs=6) so quantize is not DMA-gated.
  - x pipeline: DMA 128-token f32 tiles, DVE-cast to bf16, PE-transpose
    into xT [K, 128] (PSUM drained by DVE/ACT), then accumulate
    out[m, n] = sum_k xT[k, m] * qT[k, n] in PSUM, k-outer with both
    512-wide n-halves per stationary; the second matmul of each pair sets
    InstMatmult.ldweights=False to reuse the loaded stationary (avoids the
    PE weight-swap drain bubble). ACT copies PSUM->SBUF fused with *scale.
"""

import os
import sys

sys.path.insert(0, "/opt/trn_rl_repo")

import numpy as np

import concourse.bass as bass
import concourse.tile as tile
from concourse import bacc, mybir
from concourse.bass_utils import run_bass_kernel_spmd
from concourse.masks import make_identity
from concourse import bass_isa

F32 = mybir.dt.float32
BF16 = mybir.dt.bfloat16

NCORES = 8
M = 8192          # tokens (4*2048)
K = 2048          # in_features
N_FULL = 8192     # out_features
NS = N_FULL // NCORES  # 1024 per-core shard
P = 128
KO = K // P       # 16 k-tiles
NO = NS // P      # 8 W-row tiles per shard
MT = M // P       # 64 m-tiles
W_ELEMS = float(N_FULL * K)  # 16777216, for the mean

PREFIX = 9        # m-tiles of x work emitted before the scale collective
X_HEAD = 2        # x tiles loaded before the W pass-1 stream
REUSE_LDW = True  # second matmul of a same-stationary pair skips LDWEIGHTS


def build_nc():
    nc = bacc.Bacc("TRN2", target_bir_lowering=False, debug=False,
                   num_devices=NCORES)
    x_d = nc.dram_tensor("x", [M, K], F32, kind="ExternalInput")
    w_d = nc.dram_tensor("w", [NS, K], F32, kind="ExternalInput")
    o_d = nc.dram_tensor("out", [M, NS], F32, kind="ExternalOutput")
    x_ap, w_ap, o_ap = x_d.ap(), w_d.ap(), o_d.ap()

    with tile.TileContext(nc) as tc:
        with (
            tc.tile_pool(name="const", bufs=1) as const,
            tc.tile_pool(name="scal", bufs=1) as scal,
            tc.tile_pool(name="wkeep", bufs=4) as wkeep,
            tc.tile_pool(name="wstream", bufs=2) as wstream,
            tc.tile_pool(name="gpool", bufs=1) as gpool,
            tc.tile_pool(name="qtpool", bufs=2) as qtpool,
            tc.tile_pool(name="qT_pool", bufs=1) as qT_pool,
            tc.tile_pool(name="xpool", bufs=3) as xpool,
            tc.tile_pool(name="xbpool", bufs=3) as xbpool,
            tc.tile_pool(name="xTpool", bufs=PREFIX + 2) as xTpool,
            tc.tile_pool(name="opool", bufs=3) as opool,
            tc.tile_pool(name="psum_t", bufs=3, space="PSUM") as psum_t,
            tc.tile_pool(name="psum_o", bufs=4, space="PSUM") as psum_o,
            tc.tile_pool(name="dram", bufs=1, space="DRAM") as dram,
        ):
            ident = const.tile([P, P], BF16, name="ident")
            make_identity(nc, ident)

            # ---- x pipeline stage -------------------------------------
            def x_load(mt):
                xt = xpool.tile([P, K], F32, name=f"x_{mt}", tag="x")
                nc.sync.dma_start(xt[:], x_ap[mt * P:(mt + 1) * P, :])
                return xt

            def x_stage(mt, xt):
                xb = xbpool.tile([P, K], BF16, name=f"xb_{mt}", tag="xb")
                nc.vector.tensor_copy(xb[:], xt[:])
                xT = xTpool.tile([P, KO, P], BF16, name=f"xT_{mt}", tag="xT")
                for g in range(4):
                    pt = psum_t.tile([P, 4 * P], F32, name=f"ptx_{mt}_{g}",
                                     tag="pt")
                    for j in range(4):
                        ko = g * 4 + j
                        nc.tensor.matmul(
                            pt[:, j * P:(j + 1) * P],
                            lhsT=xb[:, ko * P:(ko + 1) * P],
                            rhs=ident[:], start=True, stop=True)
                    if g < 2:
                        nc.vector.tensor_copy(
                            xT[:, g * 4:(g + 1) * 4, :], pt[:])
                    else:
                        nc.scalar.activation(
                            xT[:, g * 4:(g + 1) * 4, :], pt[:],
                            mybir.ActivationFunctionType.Copy)
                return xT

            head = {mt: x_load(mt) for mt in range(X_HEAD)}

            # ---- W pass 1: global scale -------------------------------
            wabs = scal.tile([P, NO], F32, name="wabs")
            w_tiles = {}
            for o in range(NO):
                pool = wkeep if o < 4 else wstream
                wt = pool.tile([P, K], F32, name=f"w1_{o}",
                               tag="wk" if o < 4 else "ws")
                nc.sync.dma_start(wt[:], w_ap[o * P:(o + 1) * P, :])
                nc.vector.tensor_reduce(
                    wabs[:, o:o + 1], wt[:], mybir.AxisListType.X,
                    mybir.AluOpType.add, apply_absolute_value=True)
                w_tiles[o] = wt
            wsum = scal.tile([P, 1], F32, name="wsum")
            nc.vector.tensor_reduce(
                wsum[:], wabs[:], mybir.AxisListType.X, mybir.AluOpType.add)
            tot128 = scal.tile([P, 1], F32, name="tot128")
            nc.gpsimd.partition_all_reduce(
                tot128[:], wsum[:], P, bass_isa.ReduceOp.add)

            cc_in = dram.tile([1, 1], F32, name="cc_in")
            cc_out = dram.tile([1, 1], F32, name="cc_out", addr_space="Shared")
            nc.gpsimd.dma_start(cc_in[:], tot128[0:1, :])
            nc.gpsimd.collective_compute(
                "AllReduce", mybir.AluOpType.add,
                replica_groups=[list(range(NCORES))],
                ins=[cc_in[:].opt()], outs=[cc_out[:].opt()])
            tot_sb = scal.tile([1, 1], F32, name="tot_sb")
            nc.gpsimd.dma_start(tot_sb[:], cc_out[:])
            bcast = scal.tile([P, 1], F32, name="bcast")
            nc.gpsimd.partition_broadcast(bcast[:], tot_sb[:])

            # thr = 0.5*scale = 0.5e-4 + tot/(2*W_ELEMS); scale = 1e-4 + tot/W_ELEMS
            thr_pos = scal.tile([P, 1], F32, name="thr_pos")
            nc.vector.tensor_scalar(
                thr_pos[:], bcast[:], 0.5 / W_ELEMS, 0.5e-4,
                mybir.AluOpType.mult, mybir.AluOpType.add)
            thr_neg = scal.tile([P, 1], F32, name="thr_neg")
            nc.vector.tensor_scalar(
                thr_neg[:], thr_pos[:], -1.0, None, mybir.AluOpType.mult)
            scale_col = scal.tile([P, 1], F32, name="scale_col")
            nc.vector.tensor_scalar(
                scale_col[:], bcast[:], 1.0 / W_ELEMS, 1e-4,
                mybir.AluOpType.mult, mybir.AluOpType.add)

            # ---- x prefix (fills PE during the collective wait) -------
            prefix_xT = {}
            for mt in range(PREFIX):
                xt = head.pop(mt) if mt in head else x_load(mt)
                prefix_xT[mt] = x_stage(mt, xt)

            # ---- quantize + transpose -> qT [P, KO, NS] ---------------
            # W tiles o=0..3 stayed resident in wkeep; o=4..7 are re-read
            # through wstream on the (otherwise idle) GPSIMD ring.
            qT = qT_pool.tile([P, KO, NS], BF16, name="qT")
            for o in range(NO):
                if o >= 4:
                    wt2 = wstream.tile([P, K], F32, name=f"w2_{o}", tag="ws")
                    nc.gpsimd.dma_start(wt2[:], w_ap[o * P:(o + 1) * P, :])
                else:
                    wt2 = w_tiles[o]
                qt = qtpool.tile([P, K], BF16, name=f"qt_{o}", tag="qt")
                nc.vector.tensor_scalar(
                    qt[:], wt2[:], thr_pos[:], None, mybir.AluOpType.is_gt)
                gb = gpool.tile([P, K], BF16, name=f"gb_{o}", tag="gb")
                nc.vector.tensor_scalar(
                    gb[:], wt2[:], thr_neg[:], None, mybir.AluOpType.is_lt)
                nc.vector.tensor_tensor(
                    qt[:], qt[:], gb[:], mybir.AluOpType.subtract)
                for g in range(4):
                    pt = psum_t.tile([P, 4 * P], F32, name=f"ptq_{o}_{g}",
                                     tag="pt")
                    for j in range(4):
                        ko = g * 4 + j
                        nc.tensor.matmul(
                            pt[:, j * P:(j + 1) * P],
                            lhsT=qt[:, ko * P:(ko + 1) * P],
                            rhs=ident[:], start=True, stop=True)
                    nc.scalar.activation(
                        qT[:, g * 4:(g + 1) * 4, o * P:(o + 1) * P],
                        pt[:].rearrange("p (a b) -> p a b", a=4),
                        mybir.ActivationFunctionType.Copy)

            # ---- main loop: matmul + scale + store --------------------
            for mt in range(MT):
                if mt in prefix_xT:
                    xT = prefix_xT.pop(mt)
                else:
                    xT = x_stage(mt, x_load(mt))
                ot = opool.tile([P, NS], F32, name=f"o_{mt}", tag="o")
                pos = [psum_o.tile([P, 512], F32, name=f"po_{mt}_{nh}",
                                   tag="po") for nh in range(2)]
                for ko in range(KO):
                    mm0 = nc.tensor.matmul(
                        pos[0][:], lhsT=xT[:, ko, :],
                        rhs=qT[:, ko, 0:512],
                        start=(ko == 0), stop=(ko == KO - 1))
                    mm1 = nc.tensor.matmul(
                        pos[1][:], lhsT=xT[:, ko, :],
                        rhs=qT[:, ko, 512:1024],
                        start=(ko == 0), stop=(ko == KO - 1))
                    if REUSE_LDW:
                        mm1.ins.ldweights = False
                for nh in range(2):
                    nc.scalar.activation(
                        ot[:, nh * 512:(nh + 1) * 512], pos[nh][:],
                        mybir.ActivationFunctionType.Copy, scale=scale_col[:])
                nc.sync.dma_start(o_ap[mt * P:(mt + 1) * P, :], ot[:])

    nc.compile()
    return nc


_NC_CACHE = None


def get_nc():
    global _NC_CACHE
    if _NC_CACHE is None:
        _NC_CACHE = build_nc()
    return _NC_CACHE


def make_in_maps(x, weight):
    x2 = np.ascontiguousarray(np.asarray(x, dtype=np.float32).reshape(M, K))
    w = np.asarray(weight, dtype=np.float32)
    return [
        {"x": x2, "w": np.ascontiguousarray(w[c * NS:(c + 1) * NS])}
        for c in range(NCORES)
    ]


def kernel(x, weight):
    nc = get_nc()
    in_maps = make_in_maps(x, weight)
    try:
        res = run_bass_kernel_spmd(nc, in_maps, list(range(NCORES)))
    except Exception:
        # transient device errors have been observed on first touch; retry once
        res = run_bass_kernel_spmd(nc, in_maps, list(range(NCORES)))
    out = np.concatenate(
        [res.results[c]["out"] for c in range(NCORES)], axis=1)
    return np.ascontiguousarray(out.reshape(4, 2048, N_FULL), dtype=np.float32)


# revision 13
# speedup vs baseline: 1.0379x; 1.0022x over previous
"""BitNet linear layer (b1.58-style) on 8 Trainium2 NeuronCores.

Computes: scale = 1e-4 + mean(|W|); q = clip(round(W/scale), -1, 1);
          out = scale * (x @ q.T)
for x [4, 2048, 2048] f32 and W [8192, 2048] f32.

Sharding: tensor-parallel over out_features. Each core gets the full x
(replicated) and a 1024-row shard of W; host concatenates the 8 per-core
[8192, 1024] outputs along the feature axis.

On-device per core:
  - W pass 1 (sync/HWDGE ring, after a 2-tile x head): stream the shard,
    |W| row-sums (DVE), partition all-reduce (GPSIMD), 4-byte AllReduce
    across the 8 cores (collective plumbing alone on the GPSIMD ring)
    -> global scale. A deep prefix of x transposes keeps the PE busy for
    the whole collective wait.
  - Quantize: q = (W > .5*scale) - (W < -.5*scale), exactly equal to
    clip(round(W/s), -1, 1) for |W/s| < 2.5 with round-half-even boundary
    behavior; bf16 result is transposed 128x128 on the PE (regular matmul
    against an identity) into qT [K, N]. W is re-read for quantization
    through a 2-slot pool on the GPSIMD ring (first two tiles prefetched
    at t=0, the rest pipelined behind the collective).
  - x pipeline: DMA 128-token f32 tiles, DVE-cast to bf16, PE-transpose
    into xT [K, 128] (PSUM drained half by DVE, half by ACT), then
    accumulate out[m, n] = sum_k xT[k, m] * qT[k, n] in PSUM over 16
    k-tiles per 512-wide n-half; ACT copies PSUM->SBUF fused with *scale.
"""

import os
import sys

sys.path.insert(0, "/opt/trn_rl_repo")

import numpy as np

import concourse.bass as bass
import concourse.tile as tile
from concourse import bacc, mybir
from concourse.bass_utils import run_bass_kernel_spmd
from concourse.masks import make_identity
from concourse import bass_isa

F32 = mybir.dt.float32
BF16 = mybir.dt.bfloat16

NCORES = 8
M = 8192          # tokens (4*2048)
K = 2048          # in_features
N_FULL = 8192     # out_features
NS = N_FULL // NCORES  # 1024 per-core shard
P = 128
KO = K // P       # 16 k-tiles
NO = NS // P      # 8 W-row tiles per shard
MT = M // P       # 64 m-tiles
W_ELEMS = float(N_FULL * K)  # 16777216, for the mean

PREFIX = 17       # m-tiles of x work emitted before the scale collective
X_HEAD = 2        # x tiles loaded before the W pass-1 stream


def build_nc():
    nc = bacc.Bacc("TRN2", target_bir_lowering=False, debug=False,
                   num_devices=NCORES)
    x_d = nc.dram_tensor("x", [M, K], F32, kind="ExternalInput")
    w_d = nc.dram_tensor("w", [NS, K], F32, kind="ExternalInput")
    o_d = nc.dram_tensor("out", [M, NS], F32, kind="ExternalOutput")
    x_ap, w_ap, o_ap = x_d.ap(), w_d.ap(), o_d.ap()

    with tile.TileContext(nc) as tc:
        with (
            tc.tile_pool(name="const", bufs=1) as const,
            tc.tile_pool(name="scal", bufs=1) as scal,
            tc.tile_pool(name="wstream", bufs=2) as wstream,
            tc.tile_pool(name="w2pool", bufs=2) as w2pool,
            tc.tile_pool(name="gpool", bufs=1) as gpool,
            tc.tile_pool(name="qtpool", bufs=2) as qtpool,
            tc.tile_pool(name="qT_pool", bufs=1) as qT_pool,
            tc.tile_pool(name="xpool", bufs=2) as xpool,
            tc.tile_pool(name="xbpool", bufs=2) as xbpool,
            tc.tile_pool(name="xTpool", bufs=PREFIX + 2) as xTpool,
            tc.tile_pool(name="opool", bufs=2) as opool,
            tc.tile_pool(name="psum_t", bufs=3, space="PSUM") as psum_t,
            tc.tile_pool(name="psum_o", bufs=4, space="PSUM") as psum_o,
            tc.tile_pool(name="dram", bufs=1, space="DRAM") as dram,
        ):
            ident = const.tile([P, P], BF16, name="ident")
            make_identity(nc, ident)

            # quantize-input prefetch: first two W tiles on the (empty)
            # GPSIMD ring so quantization can start the moment the scale
            # arrives
            w2_tiles = {}
            for o in range(2):
                wt2 = w2pool.tile([P, K], F32, name=f"w2_{o}", tag="w2")
                nc.gpsimd.dma_start(wt2[:], w_ap[o * P:(o + 1) * P, :])
                w2_tiles[o] = wt2

            # ---- x pipeline stage -------------------------------------
            def x_load(mt):
                xt = xpool.tile([P, K], F32, name=f"x_{mt}", tag="x")
                nc.sync.dma_start(xt[:], x_ap[mt * P:(mt + 1) * P, :])
                return xt

            def x_stage(mt, xt):
                xb = xbpool.tile([P, K], BF16, name=f"xb_{mt}", tag="xb")
                nc.vector.tensor_copy(xb[:], xt[:])
                xT = xTpool.tile([P, KO, P], BF16, name=f"xT_{mt}", tag="xT")
                for g in range(4):
                    pt = psum_t.tile([P, 4 * P], F32, name=f"ptx_{mt}_{g}",
                                     tag="pt")
                    for j in range(4):
                        ko = g * 4 + j
                        nc.tensor.matmul(
                            pt[:, j * P:(j + 1) * P],
                            lhsT=xb[:, ko * P:(ko + 1) * P],
                            rhs=ident[:], start=True, stop=True)
                    if g < 2:
                        nc.vector.tensor_copy(
                            xT[:, g * 4:(g + 1) * 4, :], pt[:])
                    else:
                        nc.scalar.activation(
                            xT[:, g * 4:(g + 1) * 4, :], pt[:],
                            mybir.ActivationFunctionType.Copy)
                return xT

            head = {mt: x_load(mt) for mt in range(X_HEAD)}

            # ---- W pass 1: global scale -------------------------------
            wabs = scal.tile([P, NO], F32, name="wabs")
            for o in range(NO):
                wt = wstream.tile([P, K], F32, name=f"w1_{o}", tag="ws")
                nc.sync.dma_start(wt[:], w_ap[o * P:(o + 1) * P, :])
                nc.vector.tensor_reduce(
                    wabs[:, o:o + 1], wt[:], mybir.AxisListType.X,
                    mybir.AluOpType.add, apply_absolute_value=True)
            wsum = scal.tile([P, 1], F32, name="wsum")
            nc.vector.tensor_reduce(
                wsum[:], wabs[:], mybir.AxisListType.X, mybir.AluOpType.add)
            tot128 = scal.tile([P, 1], F32, name="tot128")
            nc.gpsimd.partition_all_reduce(
                tot128[:], wsum[:], P, bass_isa.ReduceOp.add)

            cc_in = dram.tile([1, 1], F32, name="cc_in")
            cc_out = dram.tile([1, 1], F32, name="cc_out", addr_space="Shared")
            nc.gpsimd.dma_start(cc_in[:], tot128[0:1, :])
            nc.gpsimd.collective_compute(
                "AllReduce", mybir.AluOpType.add,
                replica_groups=[list(range(NCORES))],
                ins=[cc_in[:].opt()], outs=[cc_out[:].opt()])
            tot_sb = scal.tile([1, 1], F32, name="tot_sb")
            nc.gpsimd.dma_start(tot_sb[:], cc_out[:])
            bcast = scal.tile([P, 1], F32, name="bcast")
            nc.gpsimd.partition_broadcast(bcast[:], tot_sb[:])

            # thr = 0.5*scale = 0.5e-4 + tot/(2*W_ELEMS); scale = 1e-4 + tot/W_ELEMS
            thr_pos = scal.tile([P, 1], F32, name="thr_pos")
            nc.vector.tensor_scalar(
                thr_pos[:], bcast[:], 0.5 / W_ELEMS, 0.5e-4,
                mybir.AluOpType.mult, mybir.AluOpType.add)
            thr_neg = scal.tile([P, 1], F32, name="thr_neg")
            nc.vector.tensor_scalar(
                thr_neg[:], thr_pos[:], -1.0, None, mybir.AluOpType.mult)
            scale_col = scal.tile([P, 1], F32, name="scale_col")
            nc.vector.tensor_scalar(
                scale_col[:], bcast[:], 1.0 / W_ELEMS, 1e-4,
                mybir.AluOpType.mult, mybir.AluOpType.add)

            # ---- x prefix (fills PE during the collective wait) -------
            prefix_xT = {}
            for mt in range(PREFIX):
                xt = head.pop(mt) if mt in head else x_load(mt)
                prefix_xT[mt] = x_stage(mt, xt)

            # ---- quantize + transpose -> qT [P, KO, NS] ---------------
            qT = qT_pool.tile([P, KO, NS], BF16, name="qT")
            for o in range(NO):
                if o in w2_tiles:
                    wt2 = w2_tiles[o]
                else:
                    wt2 = w2pool.tile([P, K], F32, name=f"w2_{o}", tag="w2")
                    nc.gpsimd.dma_start(wt2[:], w_ap[o * P:(o + 1) * P, :])
                qt = qtpool.tile([P, K], BF16, name=f"qt_{o}", tag="qt")
                nc.vector.tensor_scalar(
                    qt[:], wt2[:], thr_pos[:], None, mybir.AluOpType.is_gt)
                gb = gpool.tile([P, K], BF16, name=f"gb_{o}", tag="gb")
                nc.vector.tensor_scalar(
                    gb[:], wt2[:], thr_neg[:], None, mybir.AluOpType.is_lt)
                nc.vector.tensor_tensor(
                    qt[:], qt[:], gb[:], mybir.AluOpType.subtract)
                for g in range(4):
                    pt = psum_t.tile([P, 4 * P], F32, name=f"ptq_{o}_{g}",
                                     tag="pt")
                    for j in range(4):
                        ko = g * 4 + j
                        nc.tensor.matmul(
                            pt[:, j * P:(j + 1) * P],
                            lhsT=qt[:, ko * P:(ko + 1) * P],
                            rhs=ident[:], start=True, stop=True)
                    nc.scalar.activation(
                        qT[:, g * 4:(g + 1) * 4, o * P:(o + 1) * P],
                        pt[:].rearrange("p (a b) -> p a b", a=4),
                        mybir.ActivationFunctionType.Copy)

            # ---- main loop: matmul + scale + store --------------------
            for mt in range(MT):
                if mt in prefix_xT:
                    xT = prefix_xT.pop(mt)
                else:
                    xT = x_stage(mt, x_load(mt))
                ot = opool.tile([P, NS], F32, name=f"o_{mt}", tag="o")
                for nh in range(2):
                    po = psum_o.tile([P, 512], F32, name=f"po_{mt}_{nh}",
                                     tag="po")
                    for ko in range(KO):
                        nc.tensor.matmul(
                            po[:], lhsT=xT[:, ko, :],
                            rhs=qT[:, ko, nh * 512:(nh + 1) * 512],
                            start=(ko == 0), stop=(ko == KO - 1))
                    nc.scalar.activation(
                        ot[:, nh * 512:(nh + 1) * 512], po[:],
                        mybir.ActivationFunctionType.Copy, scale=scale_col[:])
                nc.sync.dma_start(o_ap[mt * P:(mt + 1) * P, :], ot[:])

    nc.compile()
    return nc


_NC_CACHE = None


def get_nc():
    global _NC_CACHE
    if _NC_CACHE is None:
        _NC_CACHE = build_nc()
    return _NC_CACHE


def make_in_maps(x, weight):
    x2 = np.ascontiguousarray(np.asarray(x, dtype=np.float32).reshape(M, K))
    w = np.asarray(weight, dtype=np.float32)
    return [
        {"x": x2, "w": np.ascontiguousarray(w[c * NS:(c + 1) * NS])}
        for c in range(NCORES)
    ]


def kernel(x, weight):
    nc = get_nc()
    in_maps = make_in_maps(x, weight)
    try:
        res = run_bass_kernel_spmd(nc, in_maps, list(range(NCORES)))
    except Exception:
        # transient device errors have been observed on first touch; retry once
        res = run_bass_kernel_spmd(nc, in_maps, list(range(NCORES)))
    out = np.concatenate(
        [res.results[c]["out"] for c in range(NCORES)], axis=1)
    return np.ascontiguousarray(out.reshape(4, 2048, N_FULL), dtype=np.float32)


# revision 14
# speedup vs baseline: 1.0792x; 1.0397x over previous
"""BitNet linear layer (b1.58-style) on 8 Trainium2 NeuronCores.

Computes: scale = 1e-4 + mean(|W|); q = clip(round(W/scale), -1, 1);
          out = scale * (x @ q.T)
for x [4, 2048, 2048] f32 and W [8192, 2048] f32.

Sharding: tensor-parallel over out_features. Each core gets the full x
(replicated) and a 1024-row shard of W; host concatenates the 8 per-core
[8192, 1024] outputs along the feature axis.

On-device per core:
  - W is loaded once, first in the DMA queue, as four resident 2-MiB
    pair-tiles [128, 2, 2048]. |W| row-sums (DVE) -> partition all-reduce
    (GPSIMD) -> 4-byte AllReduce across the 8 cores (collective plumbing
    alone on the GPSIMD ring) -> global scale. A deep prefix of x
    transposes keeps the PE busy during the collective wait.
  - Quantize (from the resident tiles, no second read):
    q = (W > .5*scale) - (W < -.5*scale), exactly clip(round(W/s), -1, 1)
    for |W/s| < 2.5 with round-half-even boundary behavior; bf16 result is
    transposed 128x128 on the PE (regular matmul against an identity) into
    qT [K, N].
  - x pipeline: DMA 128-token f32 tiles, DVE-cast to bf16, PE-transpose
    into xT [K, 128] (PSUM drained half by DVE, half by ACT), then
    accumulate out[m, n] = sum_k xT[k, m] * qT[k, n] in PSUM over 16
    k-tiles per 512-wide n-half; ACT copies PSUM->SBUF fused with *scale.
"""

import os
import sys

sys.path.insert(0, "/opt/trn_rl_repo")

import numpy as np

import concourse.bass as bass
import concourse.tile as tile
from concourse import bacc, mybir
from concourse.bass_utils import run_bass_kernel_spmd
from concourse.masks import make_identity
from concourse import bass_isa

F32 = mybir.dt.float32
BF16 = mybir.dt.bfloat16

NCORES = 8
M = 8192          # tokens (4*2048)
K = 2048          # in_features
N_FULL = 8192     # out_features
NS = N_FULL // NCORES  # 1024 per-core shard
P = 128
KO = K // P       # 16 k-tiles
NO = NS // P      # 8 W-row tiles per shard
MT = M // P       # 64 m-tiles
W_ELEMS = float(N_FULL * K)  # 16777216, for the mean

PREFIX = 10       # m-tiles of x work emitted before the scale collective
X_HEAD = 2        # x tiles loaded alongside the W stream


def build_nc():
    nc = bacc.Bacc("TRN2", target_bir_lowering=False, debug=False,
                   num_devices=NCORES)
    x_d = nc.dram_tensor("x", [M, K], F32, kind="ExternalInput")
    w_d = nc.dram_tensor("w", [NS, K], F32, kind="ExternalInput")
    o_d = nc.dram_tensor("out", [M, NS], F32, kind="ExternalOutput")
    x_ap, w_ap, o_ap = x_d.ap(), w_d.ap(), o_d.ap()

    with tile.TileContext(nc) as tc:
        with (
            tc.tile_pool(name="const", bufs=1) as const,
            tc.tile_pool(name="scal", bufs=1) as scal,
            tc.tile_pool(name="wpool", bufs=4) as wpool,
            tc.tile_pool(name="gpool", bufs=1) as gpool,
            tc.tile_pool(name="qtpool", bufs=2) as qtpool,
            tc.tile_pool(name="qT_pool", bufs=1) as qT_pool,
            tc.tile_pool(name="xpool", bufs=2) as xpool,
            tc.tile_pool(name="xbpool", bufs=2) as xbpool,
            tc.tile_pool(name="xTpool", bufs=PREFIX + 2) as xTpool,
            tc.tile_pool(name="opool", bufs=2) as opool,
            tc.tile_pool(name="psum_t", bufs=3, space="PSUM") as psum_t,
            tc.tile_pool(name="psum_o", bufs=4, space="PSUM") as psum_o,
            tc.tile_pool(name="dram", bufs=1, space="DRAM") as dram,
        ):
            ident = const.tile([P, P], BF16, name="ident")
            make_identity(nc, ident)

            # ---- W: one resident read, first in the DMA queue ---------
            wabs = scal.tile([P, NO], F32, name="wabs")
            w_tiles = {}
            for o2 in range(4):
                wt = wpool.tile([P, 2, K], F32, name=f"w_{o2}", tag="w")
                nc.sync.dma_start(
                    wt[:],
                    w_ap[o2 * 2 * P:(o2 + 1) * 2 * P, :].rearrange(
                        "(a p) k -> p a k", p=P))
                nc.vector.tensor_reduce(
                    wabs[:, 2 * o2:2 * o2 + 2], wt[:], mybir.AxisListType.X,
                    mybir.AluOpType.add, apply_absolute_value=True)
                w_tiles[o2] = wt

            # ---- x pipeline stage -------------------------------------
            def x_load(mt):
                xt = xpool.tile([P, K], F32, name=f"x_{mt}", tag="x")
                nc.sync.dma_start(xt[:], x_ap[mt * P:(mt + 1) * P, :])
                return xt

            def x_stage(mt, xt):
                xb = xbpool.tile([P, K], BF16, name=f"xb_{mt}", tag="xb")
                nc.vector.tensor_copy(xb[:], xt[:])
                xT = xTpool.tile([P, KO, P], BF16, name=f"xT_{mt}", tag="xT")
                for g in range(4):
                    pt = psum_t.tile([P, 4 * P], F32, name=f"ptx_{mt}_{g}",
                                     tag="pt")
                    for j in range(4):
                        ko = g * 4 + j
                        nc.tensor.matmul(
                            pt[:, j * P:(j + 1) * P],
                            lhsT=xb[:, ko * P:(ko + 1) * P],
                            rhs=ident[:], start=True, stop=True)
                    if g < 2:
                        nc.vector.tensor_copy(
                            xT[:, g * 4:(g + 1) * 4, :], pt[:])
                    else:
                        nc.scalar.activation(
                            xT[:, g * 4:(g + 1) * 4, :], pt[:],
                            mybir.ActivationFunctionType.Copy)
                return xT

            head = {mt: x_load(mt) for mt in range(X_HEAD)}

            # ---- global scale -----------------------------------------
            wsum = scal.tile([P, 1], F32, name="wsum")
            nc.vector.tensor_reduce(
                wsum[:], wabs[:], mybir.AxisListType.X, mybir.AluOpType.add)
            tot128 = scal.tile([P, 1], F32, name="tot128")
            nc.gpsimd.partition_all_reduce(
                tot128[:], wsum[:], P, bass_isa.ReduceOp.add)

            cc_in = dram.tile([1, 1], F32, name="cc_in")
            cc_out = dram.tile([1, 1], F32, name="cc_out", addr_space="Shared")
            nc.gpsimd.dma_start(cc_in[:], tot128[0:1, :])
            nc.gpsimd.collective_compute(
                "AllReduce", mybir.AluOpType.add,
                replica_groups=[list(range(NCORES))],
                ins=[cc_in[:].opt()], outs=[cc_out[:].opt()])
            tot_sb = scal.tile([1, 1], F32, name="tot_sb")
            nc.gpsimd.dma_start(tot_sb[:], cc_out[:])
            bcast = scal.tile([P, 1], F32, name="bcast")
            nc.gpsimd.partition_broadcast(bcast[:], tot_sb[:])

            # thr = 0.5*scale = 0.5e-4 + tot/(2*W_ELEMS); scale = 1e-4 + tot/W_ELEMS
            thr_pos = scal.tile([P, 1], F32, name="thr_pos")
            nc.vector.tensor_scalar(
                thr_pos[:], bcast[:], 0.5 / W_ELEMS, 0.5e-4,
                mybir.AluOpType.mult, mybir.AluOpType.add)
            thr_neg = scal.tile([P, 1], F32, name="thr_neg")
            nc.vector.tensor_scalar(
                thr_neg[:], thr_pos[:], -1.0, None, mybir.AluOpType.mult)
            scale_col = scal.tile([P, 1], F32, name="scale_col")
            nc.vector.tensor_scalar(
                scale_col[:], bcast[:], 1.0 / W_ELEMS, 1e-4,
                mybir.AluOpType.mult, mybir.AluOpType.add)

            # ---- x prefix (fills PE during the collective wait) -------
            prefix_xT = {}
            for mt in range(PREFIX):
                xt = head.pop(mt) if mt in head else x_load(mt)
                prefix_xT[mt] = x_stage(mt, xt)

            # ---- quantize + transpose -> qT [P, KO, NS] ---------------
            qT = qT_pool.tile([P, KO, NS], BF16, name="qT")
            for o in range(NO):
                wt2 = w_tiles[o // 2][:, o % 2, :]
                qt = qtpool.tile([P, K], BF16, name=f"qt_{o}", tag="qt")
                nc.vector.tensor_scalar(
                    qt[:], wt2, thr_pos[:], None, mybir.AluOpType.is_gt)
                gb = gpool.tile([P, K], BF16, name=f"gb_{o}", tag="gb")
                nc.vector.tensor_scalar(
                    gb[:], wt2, thr_neg[:], None, mybir.AluOpType.is_lt)
                nc.vector.tensor_tensor(
                    qt[:], qt[:], gb[:], mybir.AluOpType.subtract)
                for g in range(4):
                    pt = psum_t.tile([P, 4 * P], F32, name=f"ptq_{o}_{g}",
                                     tag="pt")
                    for j in range(4):
                        ko = g * 4 + j
                        nc.tensor.matmul(
                            pt[:, j * P:(j + 1) * P],
                            lhsT=qt[:, ko * P:(ko + 1) * P],
                            rhs=ident[:], start=True, stop=True)
                    nc.scalar.activation(
                        qT[:, g * 4:(g + 1) * 4, o * P:(o + 1) * P],
                        pt[:].rearrange("p (a b) -> p a b", a=4),
                        mybir.ActivationFunctionType.Copy)

            # ---- main loop: matmul + scale + store --------------------
            for mt in range(MT):
                if mt in prefix_xT:
                    xT = prefix_xT.pop(mt)
                else:
                    xT = x_stage(mt, x_load(mt))
                ot = opool.tile([P, NS], F32, name=f"o_{mt}", tag="o")
                for nh in range(2):
                    po = psum_o.tile([P, 512], F32, name=f"po_{mt}_{nh}",
                                     tag="po")
                    for ko in range(KO):
                        nc.tensor.matmul(
                            po[:], lhsT=xT[:, ko, :],
                            rhs=qT[:, ko, nh * 512:(nh + 1) * 512],
                            start=(ko == 0), stop=(ko == KO - 1))
                    nc.scalar.activation(
                        ot[:, nh * 512:(nh + 1) * 512], po[:],
                        mybir.ActivationFunctionType.Copy, scale=scale_col[:])
                nc.sync.dma_start(o_ap[mt * P:(mt + 1) * P, :], ot[:])

    nc.compile()
    return nc


_NC_CACHE = None


def get_nc():
    global _NC_CACHE
    if _NC_CACHE is None:
        _NC_CACHE = build_nc()
    return _NC_CACHE


def make_in_maps(x, weight):
    x2 = np.ascontiguousarray(np.asarray(x, dtype=np.float32).reshape(M, K))
    w = np.asarray(weight, dtype=np.float32)
    return [
        {"x": x2, "w": np.ascontiguousarray(w[c * NS:(c + 1) * NS])}
        for c in range(NCORES)
    ]


def kernel(x, weight):
    nc = get_nc()
    in_maps = make_in_maps(x, weight)
    try:
        res = run_bass_kernel_spmd(nc, in_maps, list(range(NCORES)))
    except Exception:
        # transient device errors have been observed on first touch; retry once
        res = run_bass_kernel_spmd(nc, in_maps, list(range(NCORES)))
    out = np.concatenate(
        [res.results[c]["out"] for c in range(NCORES)], axis=1)
    return np.ascontiguousarray(out.reshape(4, 2048, N_FULL), dtype=np.float32)
